# revision 34
# baseline (speedup 1.0000x reference)
"""Trainium2 Bass kernel for nn_ContinuousThoughtBlock.

Strategy: data-parallel over batch (B=8 -> 8 NeuronCores) for the heavy
per-batch work (context mean, gate matmul, final LN), plus an H-sharded
cooperative thought phase:

  - every core evolves ALL 64 (path, batch) vectors through the 4
    residual-MLP steps, but only for ITS 512-wide slice of the hidden
    dim H=4096 (host hands core r only W1[:, r*512:(r+1)*512] and
    W2[r*512:(r+1)*512, :]); a per-step AllReduce sums the dense2
    partials.  This streams N=64 columns per 128x128 PE weight tile
    instead of N=8, cutting the PE weight-load cost ~8x.
  - one AllGather shares the (normalized) context d-major so every core
    can seed all batches' thoughts; a per-core one-hot `bsel` input
    selects the core's own batch at collapse time (SPMD graph is
    identical across cores, only data differs).
  - all 16 gate tiles run up front (their PE work hides the AllGather +
    cross-core launch skew); gate stays in SBUF.
  - scalar-engine ACT table switches are minimized (Sigmoid up front,
    one Abs_reciprocal_sqrt + one Gelu_apprx_tanh set per step, one Exp
    for the softmax, one batched Abs_reciprocal_sqrt in the final LN).
  - DMA queues: sync HWDGE = h + hT transposes + AR bounces + output;
    scalar HWDGE = weights; gpsimd = collectives + accumulate-DMAs.
"""

import numpy as np

import concourse.bass as bass
import concourse.mybir as mybir
import concourse.tile as tile
from concourse import bacc
from concourse.bass_utils import run_bass_kernel_spmd
from concourse.masks import make_identity

# Problem constants (hardcoded per harness contract).
B, L, D, H = 8, 2048, 1024, 4096
NUM_PATHS = 8
NUM_STEPS = 4
PRUNE = 0.1
EPS = 1e-6
KD = D // 128     # 8  D-chunks
ML = L // 128     # 16 L-tiles
HSH = H // B      # 512 per-core H slice
KH = HSH // 128   # 4  H-slice chunks
V = NUM_PATHS * B  # 64 vectors; pq index = q*8 + b (b innermost)
INV_SQRT_D = 1.0 / float(np.sqrt(np.float32(D)))

F32 = mybir.dt.float32
BF16 = mybir.dt.bfloat16
AF = mybir.ActivationFunctionType
ALU = mybir.AluOpType
AX = mybir.AxisListType

RG = [list(range(B))]


def _bc0(ap, n=128):
    """Broadcast a 1-D AP down n partitions via a stride-0 partition dim."""
    return bass.AP(tensor=ap.tensor, offset=ap.offset, ap=[[0, n]] + list(ap.ap))


def _rep0(ap, n, pos=1):
    """Insert a stride-0 free dim of extent n at position pos."""
    new = list(ap.ap)
    new.insert(pos, [0, n])
    return bass.AP(tensor=ap.tensor, offset=ap.offset, ap=new)


def build_graph(triv, debug=False):
    nc = bacc.Bacc("TRN2", target_bir_lowering=False, debug=False,
                   enable_asserts=True, num_devices=B)

    h_ext = nc.declare_dram_parameter("hidden_states", [L, D], BF16, isOutput=False)
    w = {}
    w["aggregator_weight"] = nc.declare_dram_parameter(
        "aggregator_weight", [D, D], BF16, isOutput=False)
    w["projector_dense1_weight"] = nc.declare_dram_parameter(
        "projector_dense1_weight", [D, HSH], BF16, isOutput=False)
    w["projector_dense2_weight"] = nc.declare_dram_parameter(
        "projector_dense2_weight", [HSH, D], BF16, isOutput=False)
    w["broadcast_weight"] = nc.declare_dram_parameter(
        "broadcast_weight", [D, D], BF16, isOutput=False)
    w["gate_weight"] = nc.declare_dram_parameter(
        "gate_weight", [D, D], BF16, isOutput=False)
    for n in ("input_norm_gamma", "input_norm_beta", "aggregator_bias",
              "projector_norm_gamma", "projector_norm_beta",
              "projector_dense1_bias", "projector_dense2_bias",
              "broadcast_bias", "gate_bias",
              "output_norm_gamma", "output_norm_beta"):
        shape = [HSH] if n == "projector_dense1_bias" else [D]
        w[n] = nc.declare_dram_parameter(n, shape, F32, isOutput=False)
    w["bsel"] = nc.declare_dram_parameter("bsel", [B], F32, isOutput=False)
    out_ext = nc.declare_dram_parameter("out", [L, D], BF16, isOutput=True)
    dbg = {}
    if debug:
        for nm, shape in (("d_ctxT", [128, KD]), ("d_ctxnT", [128, KD * B]),
                          ("d_th0", [128, KD * B]), ("d_tT", [128, KD * V]),
                          ("d_scores", [1, NUM_PATHS]), ("d_amps", [1, NUM_PATHS]),
                          ("d_final", [128, KD]), ("d_bcbf", [128, D]),
                          ("d_gate0", [128, D]), ("d_y0", [128, KD * V])):
            dbg[nm] = nc.declare_dram_parameter(nm, shape, F32, isOutput=True)

    with tile.TileContext(nc) as tc:
        _build_body(nc, tc, h_ext, w, out_ext, triv, dbg)
    nc.compile()
    return nc


def _dmajor(nc, pool, ps_pool, ident_bf, dram_ap, n, name):
    """DMA a [n*128] DRAM vector into a [128, n] d-major SBUF tile
    (tile[p, k] = v[k*128 + p]) via a bf16 [n,128] load + PE transpose."""
    rowk = pool.tile([n, 128], BF16, tag="dmaj_rowk")
    nc.gpsimd.dma_start(out=rowk[:], in_=dram_ap.rearrange("(k p) -> k p", p=128))
    ps = ps_pool.tile([128, n], BF16, tag="tr")
    nc.tensor.transpose(ps[:], rowk[:], ident_bf[0:n, 0:n])
    t = pool.tile([128, n], F32, tag=name)
    nc.scalar.copy(t[:], ps[:])
    return t


def _build_body(nc, tc, h_ext, w, out_ext, triv, dbg=None):
    dbg = dbg or {}
    import contextlib
    ctx = contextlib.ExitStack()
    with ctx:
        # ---------------- pools ----------------
        singles = ctx.enter_context(tc.tile_pool(name="singles", bufs=1))
        smalls = ctx.enter_context(tc.tile_pool(name="smalls", bufs=1))
        tstate = ctx.enter_context(tc.tile_pool(name="tstate", bufs=2))
        hTm_pool = ctx.enter_context(tc.tile_pool(name="hTm", bufs=2))
        fin = ctx.enter_context(tc.tile_pool(name="fin", bufs=3))
        wpool = ctx.enter_context(tc.tile_pool(name="wpool", bufs=1))
        dram = ctx.enter_context(tc.tile_pool(name="dram", bufs=2, space="DRAM"))

        ps_small = ctx.enter_context(tc.tile_pool(name="ps_small", bufs=1, space="PSUM"))
        ps_gate = ctx.enter_context(tc.tile_pool(name="ps_gate", bufs=2, space="PSUM"))
        ps_th = ctx.enter_context(tc.tile_pool(name="ps_th", bufs=1, space="PSUM"))

        need_ident = not (triv["aggregator_bias"] and triv["projector_norm"]
                          and triv["projector_dense1_bias"]
                          and triv["projector_dense2_bias"])

        # ---------------- constants ----------------
        eps1 = singles.tile([1, 1], F32)
        nc.vector.memset(eps1[:], EPS)
        # preload the one ACT table set (natural_log_exp) on empty DMA rings
        dummy = smalls.tile([1, 1], F32, tag="dummy")
        nc.scalar.activation(dummy[:], eps1[0:1, :], AF.Exp)
        ident_bf = None
        ps_tr = None
        if need_ident:
            ident_bf = singles.tile([128, 128], BF16)
            make_identity(nc, ident_bf[:])
            ps_tr = ctx.enter_context(tc.tile_pool(name="ps_tr", bufs=1, space="PSUM"))
        ones_bf = singles.tile([128, 1], BF16)
        nc.vector.memset(ones_bf[:], 1.0)
        ones_f32 = singles.tile([128, 1], F32)
        nc.vector.memset(ones_f32[:], 1.0)
        ones_row = singles.tile([1, 128], F32)
        nc.vector.memset(ones_row[:], 1.0)
        ones_row_bf = singles.tile([1, 128], BF16)
        nc.vector.memset(ones_row_bf[:], 1.0)
        eps_col = singles.tile([128, 1], F32)
        nc.vector.memset(eps_col[:], EPS)

        # resident tensors
        h_bf = singles.tile([128, ML, D], BF16)        # 32KB/part
        gate_bf = singles.tile([128, ML, D], BF16)     # 32KB/part
        w1_bf = wpool.tile([128, KD, HSH], BF16)       # 8KB/part
        w2_bf = wpool.tile([128, KH, D], BF16)         # 8KB/part
        wg_bf = wpool.tile([128, KD, D], BF16)         # 16KB/part
        wab_bf = wpool.tile([128, KD, D], BF16)        # 16KB/part (Wagg)
        wbc_bf = wpool.tile([128, KD, D], BF16)        # 16KB/part (Wbc)

        # d-major bias/gamma vectors (only when nontrivial)
        gammaT_pr = betaT_pr = None
        if not triv["projector_norm"]:
            gammaT_pr = _dmajor(nc, singles, ps_tr, ident_bf,
                                w["projector_norm_gamma"].ap(), KD, "g_pr")
            betaT_pr = _dmajor(nc, singles, ps_tr, ident_bf,
                               w["projector_norm_beta"].ap(), KD, "b_pr")
        baggT = None
        if not triv["aggregator_bias"]:
            baggT = _dmajor(nc, singles, ps_tr, ident_bf,
                            w["aggregator_bias"].ap(), KD, "bagg")
        b1T = None
        if not triv["projector_dense1_bias"]:
            b1T = _dmajor(nc, singles, ps_tr, ident_bf,
                          w["projector_dense1_bias"].ap(), KH, "b1")
        b2T_rep = None
        if not triv["projector_dense2_bias"]:
            b2T = _dmajor(nc, singles, ps_tr, ident_bf,
                          w["projector_dense2_bias"].ap(), KD, "b2")
            b2T_rep = _rep0(b2T[:], V, pos=2)  # [128, KD, V] view
        gbias_row = None
        if not triv["gate_bias"]:
            gbias_row = smalls.tile([1, D], BF16, tag="brow")
            nc.gpsimd.dma_start(out=gbias_row[:],
                                in_=w["gate_bias"].ap().rearrange("(a d) -> a d", a=1))

        # ---------------- h load (sync HWDGE) ----------------
        h_src = h_ext.ap().rearrange("(m t p) d -> p m t d", p=128, t=2)
        for m2 in range(ML // 2):
            nc.sync.dma_start(out=h_bf[:, 2 * m2:2 * m2 + 2, :], in_=h_src[:, m2])

        # ---------------- weight loads (scalar HWDGE) ----------------
        wg_src = w["gate_weight"].ap().rearrange("(k t p) d -> p k t d", p=128, t=2)
        for k2 in range(KD // 2):
            nc.scalar.dma_start(out=wg_bf[:, 2 * k2:2 * k2 + 2, :], in_=wg_src[:, k2])
        wagg_src = w["aggregator_weight"].ap().rearrange("(k t p) d -> p k t d",
                                                         p=128, t=2)
        for k2 in range(KD // 2):
            nc.scalar.dma_start(out=wab_bf[:, 2 * k2:2 * k2 + 2, :],
                                in_=wagg_src[:, k2])
        w1_src = w["projector_dense1_weight"].ap().rearrange(
            "(k t p) s -> p k t s", p=128, t=2)
        for k2 in range(KD // 2):
            nc.scalar.dma_start(out=w1_bf[:, 2 * k2:2 * k2 + 2, :], in_=w1_src[:, k2])
        w2_src = w["projector_dense2_weight"].ap().rearrange(
            "(k t p) d -> p k t d", p=128, t=2)
        for k2 in range(KH // 2):
            nc.scalar.dma_start(out=w2_bf[:, 2 * k2:2 * k2 + 2, :], in_=w2_src[:, k2])
        wbc_src = w["broadcast_weight"].ap().rearrange("(k t p) d -> p k t d",
                                                       p=128, t=2)
        for k2 in range(KD // 2):
            nc.scalar.dma_start(out=wbc_bf[:, 2 * k2:2 * k2 + 2, :],
                                in_=wbc_src[:, k2])

        # ---------------- context mean (stream h through ones) -----------
        # two [1, 512] row accumulators (separate banks), m-outer so the
        # streams chase the h load; then 8 tiny K=1 matmuls spread the row
        # d-major.
        ctx_r0 = ps_th.tile([1, 512], F32, tag="stt")
        ctx_r1 = ps_th.tile([1, 512], F32, tag="sts")
        for m in range(ML):
            nc.tensor.matmul(ctx_r0[0:1, :], ones_bf[:], h_bf[:, m, 0:512],
                             start=(m == 0), stop=(m == ML - 1))
            nc.tensor.matmul(ctx_r1[0:1, :], ones_bf[:], h_bf[:, m, 512:D],
                             start=(m == 0), stop=(m == ML - 1))
        ctx_row = smalls.tile([1, D], BF16, tag="ctxrow")
        nc.vector.tensor_scalar(ctx_row[0:1, 0:512], ctx_r0[0:1, :], 1.0 / L,
                                None, op0=ALU.mult)
        nc.vector.tensor_scalar(ctx_row[0:1, 512:D], ctx_r1[0:1, :], 1.0 / L,
                                None, op0=ALU.mult)
        ctx_cols = ps_th.tile([128, KD, B], F32, tag="th0ps")
        for k in range(KD):
            nc.tensor.matmul(ctx_cols[:, k, 0:1],
                             ctx_row[0:1, k * 128:(k + 1) * 128],
                             ones_bf[0:1, :], start=True, stop=True)
        agin_sb = smalls.tile([128, KD], BF16, tag="aginsb")
        nc.vector.tensor_copy(agin_sb[:], ctx_cols[:, :, 0])

        # ---------------- AllGather raw context (absorbs skew) ------------
        agin = dram.tile([128, KD], BF16, tag="agin")
        agout = dram.tile([128 * B, KD], BF16, tag="agout", addr_space="Shared")
        nc.sync.dma_start(out=agin[:], in_=agin_sb[:])
        nc.gpsimd.collective_compute(
            "AllGather", ALU.bypass, replica_groups=RG,
            ins=[agin.opt()], outs=[agout.opt()])
        # bulk readback: rows_all[p, b, k] = agout[b*128+p, k]  (bf16)
        rows_all = smalls.tile([128, B, KD], BF16, tag="rowsall")
        nc.gpsimd.dma_start(
            out=rows_all[:],
            in_=agout[:].rearrange("(b p) j -> p b j", p=128))

        # ---------------- gate tiles (hide AG + skew) ---------------------
        # sigmoid via exp so the ACT table set never changes:
        #   sigma(x) = 1 / (1 + exp(-x))
        def emit_gate_tiles(ms):
            for m in ms:
                hTm = hTm_pool.tile([128, KD, 128], BF16, tag="hTm")
                nc.scalar.dma_start_transpose(out=hTm[:], in_=h_bf[:, m, :])
                for k in range(KD):
                    for n in range(2):
                        if k == 0:
                            g_ps = ps_gate.tile([128, 512], F32, tag="gps",
                                                name=f"g_ps_{m}_{n}")
                            if n == 0:
                                g_ps0 = g_ps
                            else:
                                g_ps1 = g_ps
                        g_ps = g_ps0 if n == 0 else g_ps1
                        nc.tensor.matmul(g_ps[:], hTm[:, k, :],
                                         wg_bf[:, k, n * 512:(n + 1) * 512],
                                         start=(k == 0),
                                         stop=(k == KD - 1 and gbias_row is None))
                if gbias_row is not None:
                    for n, g_ps in ((0, g_ps0), (1, g_ps1)):
                        nc.tensor.matmul(g_ps[:], ones_row_bf[:],
                                         gbias_row[0:1, n * 512:(n + 1) * 512],
                                         start=False, stop=True)
                for n, g_ps in ((0, g_ps0), (1, g_ps1)):
                    gsc = smalls.tile([128, 512], F32, tag="gsc")
                    nc.scalar.activation(gsc[:], g_ps[:], AF.Exp, scale=-1.0)
                    gsc2 = smalls.tile([128, 512], F32, tag="gsc2")
                    nc.vector.tensor_scalar(gsc2[:], gsc[:], 1.0, None, op0=ALU.add)
                    with nc.allow_low_precision(reason="gate in (0,1), bf16 ok"):
                        nc.vector.reciprocal(gate_bf[:, m, n * 512:(n + 1) * 512],
                                             gsc2[:])

        emit_gate_tiles(range(0, 12))

        # ---------------- post-AG: LN(ctx) for all batches ----------------
        sq_rows = smalls.tile([128, B, KD], F32, tag="sqrows")
        nc.vector.tensor_mul(sq_rows[:], rows_all[:], rows_all[:])
        cst = ps_th.tile([1, 512], F32, tag="stt")
        css = ps_th.tile([1, 512], F32, tag="sts")
        for k in range(KD):
            nc.tensor.matmul(cst[0:1, 0:B], ones_bf[:], rows_all[:, :, k],
                             start=(k == 0), stop=(k == KD - 1))
        for k in range(KD):
            nc.tensor.matmul(css[0:1, 0:B], ones_f32[:], sq_rows[:, :, k],
                             start=(k == 0), stop=(k == KD - 1))
        cms = smalls.tile([1, 2 * B], F32, tag="cms")   # [mean | rstd]
        nc.vector.tensor_scalar(cms[0:1, 0:B], cst[0:1, 0:B], 1.0 / D, None,
                                op0=ALU.mult)
        cm2 = smalls.tile([1, B], F32, tag="cm2")
        nc.vector.tensor_mul(cm2[0:1, :], cms[0:1, 0:B], cms[0:1, 0:B])
        cvar = smalls.tile([1, B], F32, tag="cvar")
        nc.vector.scalar_tensor_tensor(cvar[0:1, :], css[0:1, 0:B], 1.0 / D,
                                       cm2[0:1, :], op0=ALU.mult,
                                       op1=ALU.subtract)
        clv = smalls.tile([1, B], F32, tag="clv")
        nc.scalar.activation(clv[0:1, :], cvar[0:1, :], AF.Ln, bias=eps1[0:1, :])
        nc.scalar.activation(cms[0:1, B:], clv[0:1, :], AF.Exp, scale=-0.5)
        cmr_t = ps_small.tile([128, 2 * V], F32, tag="smb")
        nc.tensor.matmul(cmr_t[:, 0:2 * B], ones_row[:], cms[:], start=True,
                         stop=True)
        ctxn_f = smalls.tile([128, KD, B], F32, tag="ctxnf")
        rows_kb = bass.AP(tensor=rows_all[:].tensor, offset=rows_all[:].offset,
                          ap=[list(rows_all[:].ap)[0], [1, KD], [KD, B]])
        nc.vector.tensor_tensor(out=ctxn_f[:], in0=rows_kb,
                                in1=_rep0(cmr_t[:, 0:B], KD), op=ALU.subtract)
        ctxn_bf = smalls.tile([128, KD, B], BF16, tag="ctxnbf")
        nc.vector.tensor_tensor(out=ctxn_bf[:], in0=ctxn_f[:],
                                in1=_rep0(cmr_t[:, B:2 * B], KD), op=ALU.mult)
        if not triv["input_norm"]:
            gin = _dmajor(nc, singles, ps_tr, ident_bf,
                          w["input_norm_gamma"].ap(), KD, "g_in")
            bin_ = _dmajor(nc, singles, ps_tr, ident_bf,
                           w["input_norm_beta"].ap(), KD, "b_in")
            nc.vector.tensor_tensor(out=ctxn_f[:], in0=ctxn_bf[:],
                                    in1=_rep0(gin[:], B, pos=2), op=ALU.mult)
            nc.vector.tensor_tensor(out=ctxn_bf[:], in0=ctxn_f[:],
                                    in1=_rep0(bin_[:], B, pos=2), op=ALU.add)
        if "d_ctxnT" in dbg:
            nc.gpsimd.dma_start(out=dbg["d_ctxnT"].ap(),
                                in_=ctxn_bf[:].rearrange("a k b -> a (k b)"))
        if "d_ctxT" in dbg:
            nc.gpsimd.dma_start(out=dbg["d_ctxT"].ap(), in_=agin_sb[:])

        # ---------------- th0 for all batches ----------------
        th0 = smalls.tile([128, KD, B], F32, tag="th0")
        thp = ps_th.tile([128, KD, B], F32, tag="th0ps")
        for dm in range(KD):
            for k in range(KD):
                nc.tensor.matmul(thp[:, dm, :],
                                 wab_bf[:, k, dm * 128:(dm + 1) * 128],
                                 ctxn_bf[:, k, :],
                                 start=(k == 0), stop=(k == KD - 1))
        nc.vector.tensor_copy(th0[:], thp[:])
        if baggT is not None:
            nc.vector.tensor_tensor(out=th0[:], in0=th0[:],
                                    in1=_rep0(baggT[:], B, pos=2), op=ALU.add)
        if "d_th0" in dbg:
            nc.sync.dma_start(out=dbg["d_th0"].ap(),
                              in_=th0[:].rearrange("a k b -> a (k b)"))
        # seed 64 paths: tT[p, k, q*8+b] = th0[p, k, b] * (1 + 0.02 q)
        tT = tstate.tile([128, KD, V], F32, tag="tT")
        for q in range(NUM_PATHS):
            nc.vector.tensor_scalar(tT[:, :, q * B:(q + 1) * B], th0[:],
                                    1.0 + 0.02 * q, None, op0=ALU.mult)

        # bsel + ctxsel (raw ctx, bf16 ok for scores)
        bsel_row = smalls.tile([1, B], F32, tag="bselr")
        nc.sync.dma_start(out=bsel_row[:],
                          in_=w["bsel"].ap().rearrange("(a b) -> a b", a=1))
        bsel_sb = smalls.tile([128, B], F32, tag="bselsb")
        nc.sync.dma_start(out=bsel_sb[:], in_=_bc0(w["bsel"].ap()))
        ctxsel = smalls.tile([128, KD, B], F32, tag="ctxsel")
        nc.vector.tensor_tensor(out=ctxsel[:], in0=rows_kb,
                                in1=_rep0(bsel_sb[:], KD, pos=1), op=ALU.mult)

        # column sums of W1 slice (for LN-mean correction after dense1)
        c1r = ps_th.tile([1, 512], F32, tag="stt")
        for k in range(KD):
            nc.tensor.matmul(c1r[0:1, :], ones_bf[:], w1_bf[:, k, :],
                             start=(k == 0), stop=(k == KD - 1))
        c1row = smalls.tile([1, HSH], BF16, tag="c1row")
        nc.vector.tensor_copy(c1row[0:1, :], c1r[0:1, :])
        c1cols = ps_th.tile([128, KD, B], F32, tag="th0ps")
        for hc in range(KH):
            nc.tensor.matmul(c1cols[:, hc, 0:1],
                             c1row[0:1, hc * 128:(hc + 1) * 128],
                             ones_bf[0:1, :], start=True, stop=True)
        colW1 = smalls.tile([128, KH], F32, tag="colW1")
        nc.vector.tensor_copy(colW1[:], c1cols[:, 0:KH, 0])

        # ---------------- thought steps (H-sharded + AllReduce) ----------
        _gate_slices = {0: range(12, 14), 1: range(14, 16)}
        GELU_SCALE = 2.0 * 0.7978845608028654
        for step in range(NUM_STEPS):
            # dense1 on RAW t (mean/rstd fixed up afterwards)
            tn_bf = smalls.tile([128, KD, V], BF16, tag="tnbf")
            nc.vector.tensor_copy(tn_bf[:], tT[:])
            # LN stats in parallel: k-accumulated ones-matmuls -> [1, V]
            sq = smalls.tile([128, KD, V], F32, tag="sq")
            nc.vector.tensor_mul(sq[:], tT[:], tT[:])
            st_t = ps_th.tile([1, 512], F32, tag="stt")
            st_s = ps_th.tile([1, 512], F32, tag="sts")
            for k in range(KD):
                nc.tensor.matmul(st_t[0:1, 0:V], ones_f32[:], tT[:, k, :],
                                 start=(k == 0), stop=(k == KD - 1))
            for k in range(KD):
                nc.tensor.matmul(st_s[0:1, 0:V], ones_f32[:], sq[:, k, :],
                                 start=(k == 0), stop=(k == KD - 1))
            ms = smalls.tile([1, 2 * V], F32, tag="ms")  # [mean | rstd]
            nc.vector.tensor_scalar(ms[0:1, 0:V], st_t[0:1, 0:V], 1.0 / D, None,
                                    op0=ALU.mult)
            m2 = smalls.tile([1, V], F32, tag="m2")
            nc.vector.tensor_mul(m2[0:1, :], ms[0:1, 0:V], ms[0:1, 0:V])
            var = smalls.tile([1, V], F32, tag="var")
            nc.vector.scalar_tensor_tensor(var[0:1, :], st_s[0:1, 0:V], 1.0 / D,
                                           m2[0:1, :], op0=ALU.mult,
                                           op1=ALU.subtract)
            lv = smalls.tile([1, V], F32, tag="lv")
            nc.scalar.activation(lv[0:1, :], var[0:1, :], AF.Ln, bias=eps1[0:1, :])
            nc.scalar.activation(ms[0:1, V:], lv[0:1, :], AF.Exp, scale=-0.5)
            mr_t = ps_small.tile([128, 2 * V], F32, tag="smb")
            nc.tensor.matmul(mr_t[:], ones_row[:], ms[:], start=True, stop=True)

            # dense1 matmuls (raw t)
            x1_ps = ps_th.tile([128, KH, V], F32, tag="x1ps")
            for hc in range(KH):
                for k in range(KD):
                    nc.tensor.matmul(x1_ps[:, hc, :],
                                     w1_bf[:, k, hc * 128:(hc + 1) * 128],
                                     tn_bf[:, k, :],
                                     start=(k == 0), stop=(k == KD - 1))
            # fix: x = (x1_raw - mean_v * colW1 [+ b1]) * rstd_v
            cfix = smalls.tile([128, KH, V], F32, tag="cfix")
            nc.vector.tensor_tensor(out=cfix[:],
                                    in0=_rep0(mr_t[:, 0:V], KH),
                                    in1=_rep0(colW1[:], V, pos=2), op=ALU.mult)
            xg = smalls.tile([128, KH, V], F32, tag="xg")
            nc.vector.tensor_sub(xg[:], x1_ps[:], cfix[:])
            if b1T is not None:
                nc.vector.tensor_tensor(out=xg[:], in0=xg[:],
                                        in1=_rep0(b1T[:], V, pos=2), op=ALU.add)
            x = smalls.tile([128, KH, V], F32, tag="xx")
            nc.vector.tensor_tensor(out=x[:], in0=xg[:],
                                    in1=_rep0(mr_t[:, V:], KH), op=ALU.mult)
            # gelu(x) = x * (1 - 1/(1 + exp(2*0.79788*(x + 0.044715 x^3))))
            sqx = smalls.tile([128, KH, V], F32, tag="sqx")
            nc.vector.tensor_mul(sqx[:], x[:], x[:])
            zf = smalls.tile([128, KH, V], F32, tag="zf")
            nc.vector.tensor_scalar(zf[:], sqx[:], 0.044715, 1.0,
                                    op0=ALU.mult, op1=ALU.add)
            z = smalls.tile([128, KH, V], F32, tag="zz")
            nc.vector.tensor_mul(z[:], zf[:], x[:])
            e = smalls.tile([128, KH, V], F32, tag="ee")
            nc.scalar.activation(e[:], z[:], AF.Exp, scale=GELU_SCALE)
            ep = smalls.tile([128, KH, V], F32, tag="ep")
            nc.vector.tensor_scalar(ep[:], e[:], 1.0, None, op0=ALU.add)
            rg = smalls.tile([128, KH, V], F32, tag="rg")
            nc.vector.reciprocal(rg[:], ep[:])
            om = smalls.tile([128, KH, V], F32, tag="om")
            nc.vector.tensor_scalar(om[:], rg[:], -1.0, 1.0,
                                    op0=ALU.mult, op1=ALU.add)
            x1_bf = smalls.tile([128, KH, V], BF16, tag="x1bf")
            nc.vector.tensor_mul(x1_bf[:], om[:], x[:])

            # dense2 partial (H slice)
            y_ps = ps_th.tile([128, KD, V], F32, tag="yps")
            for dm in range(KD):
                for hk in range(KH):
                    nc.tensor.matmul(y_ps[:, dm, :],
                                     w2_bf[:, hk, dm * 128:(dm + 1) * 128],
                                     x1_bf[:, hk, :],
                                     start=(hk == 0), stop=(hk == KH - 1))
            y_sb = smalls.tile([128, KD, V], F32, tag="ysb")
            nc.vector.tensor_copy(y_sb[:], y_ps[:])

            # AllReduce dense2 partials; th update via accumulate-DMA
            arin = dram.tile([128, KD * V], F32, tag="arin")
            arout = dram.tile([128, KD * V], F32, tag="arout", addr_space="Shared")
            nc.sync.dma_start(out=arin[:],
                              in_=y_sb[:].rearrange("a k v -> a (k v)"))
            nc.gpsimd.collective_compute(
                "AllReduce", ALU.add, replica_groups=RG,
                ins=[arin.opt()], outs=[arout.opt()])
            tT_new = tstate.tile([128, KD, V], F32, tag="tT")
            if b2T_rep is not None:
                nc.vector.tensor_tensor(out=tT_new[:], in0=tT[:], in1=b2T_rep,
                                        op=ALU.add)
            else:
                nc.vector.tensor_copy(tT_new[:], tT[:])
            if step in _gate_slices:
                emit_gate_tiles(_gate_slices[step])
            nc.gpsimd.dma_start(out=tT_new[:].rearrange("a k v -> a (k v)"),
                                in_=arout[:], accum_op=ALU.add)
            if step == 0 and "d_y0" in dbg:
                yr_dbg = smalls.tile([128, KD * V], F32, tag="yrdbg")
                nc.sync.dma_start(out=yr_dbg[:], in_=arout[:])
                nc.sync.dma_start(out=dbg["d_y0"].ap(), in_=yr_dbg[:])
            tT = tT_new

        if "d_tT" in dbg:
            nc.sync.dma_start(out=dbg["d_tT"].ap(),
                              in_=tT[:].rearrange("a k v -> a (k v)"))

        # ---------------- scores / softmax / prune (own batch) -----------
        prod = smalls.tile([128, KD, V], F32, tag="prodsc")
        nc.vector.tensor_tensor(
            out=prod[:].rearrange("a k (q b) -> a k q b", b=B),
            in0=tT[:].rearrange("a k (q b) -> a k q b", b=B),
            in1=_rep0(ctxsel[:], NUM_PATHS, pos=2),
            op=ALU.mult)
        sc_ps = ps_th.tile([1, 512], F32, tag="stt")
        for k in range(KD):
            nc.tensor.matmul(sc_ps[0:1, 0:V], ones_f32[:], prod[:, k, :],
                             start=(k == 0), stop=(k == KD - 1))
        sc = smalls.tile([1, NUM_PATHS], F32, tag="sc")
        nc.vector.tensor_reduce(sc[0:1, :],
                                sc_ps[0:1, 0:V].rearrange("a (q b) -> a q b", b=B),
                                axis=AX.X, op=ALU.add)
        # scores are O(1); skip max-subtraction in the softmax
        esum = smalls.tile([1, 1], F32, tag="esum")
        ex = smalls.tile([1, NUM_PATHS], F32, tag="ex")
        nc.scalar.activation(ex[:], sc[:], AF.Exp, scale=INV_SQRT_D,
                             accum_out=esum[:])
        if "d_scores" in dbg:
            scd = smalls.tile([1, NUM_PATHS], F32, tag="scd")
            nc.vector.tensor_scalar(scd[:], sc[:], INV_SQRT_D, None, op0=ALU.mult)
            nc.sync.dma_start(out=dbg["d_scores"].ap(), in_=scd[:])
        rsum = smalls.tile([1, 1], F32, tag="rsum")
        nc.vector.reciprocal(rsum[:], esum[:])
        amps0 = smalls.tile([1, NUM_PATHS], F32, tag="amps0")
        nc.vector.tensor_scalar(amps0[:], ex[:], rsum[0:1, :], None, op0=ALU.mult)
        mask = smalls.tile([1, NUM_PATHS], F32, tag="mask")
        nc.vector.tensor_scalar(mask[:], amps0[:], PRUNE, None, op0=ALU.is_ge)
        pruned = smalls.tile([1, NUM_PATHS], F32, tag="pruned")
        nc.vector.tensor_mul(pruned[:], amps0[:], mask[:])
        psum_s = smalls.tile([1, 1], F32, tag="psums")
        nc.vector.tensor_reduce(psum_s[:], pruned[:], axis=AX.X, op=ALU.add)
        nc.vector.tensor_scalar(psum_s[:], psum_s[:], EPS, None, op0=ALU.add)
        rr = smalls.tile([1, 1], F32, tag="rr")
        nc.vector.reciprocal(rr[:], psum_s[:])
        ampsF = smalls.tile([1, NUM_PATHS], F32, tag="ampsF")
        nc.vector.tensor_scalar(ampsF[:], pruned[:], rr[0:1, :], None, op0=ALU.mult)
        if "d_amps" in dbg:
            nc.sync.dma_start(out=dbg["d_amps"].ap(), in_=ampsF[:])

        # ampsel[1, q*8+b] = ampsF[q] * bsel[b]
        ampsel = smalls.tile([1, V], F32, tag="ampsel")
        nc.vector.tensor_tensor(
            out=ampsel[0:1, :].rearrange("a (q b) -> a q b", b=B),
            in0=_rep0(ampsF[0:1, :], B, pos=2),
            in1=_rep0(bsel_row[0:1, :], NUM_PATHS, pos=1),
            op=ALU.mult)

        # ---------------- collapse + bc ----------------
        ab_t = ps_small.tile([128, 2 * V], F32, tag="smb")
        nc.tensor.matmul(ab_t[:, 0:V], ones_row[:], ampsel[:], start=True, stop=True)
        prod2 = smalls.tile([128, KD, V], F32, tag="prod2")
        nc.vector.tensor_tensor(out=prod2[:], in0=tT[:],
                                in1=_rep0(ab_t[:, 0:V], KD), op=ALU.mult)
        finalT = smalls.tile([128, KD], F32, tag="finalT")
        nc.vector.tensor_reduce(finalT[:], prod2[:], axis=AX.X, op=ALU.add)
        if "d_final" in dbg:
            nc.sync.dma_start(out=dbg["d_final"].ap(), in_=finalT[:])
        finalT_bf = smalls.tile([128, KD], BF16, tag="finalTbf")
        nc.vector.tensor_copy(finalT_bf[:], finalT[:])

        # bc row [1, D] in two 512-halves, then K=1 matmul broadcast
        bc_bf = singles.tile([128, D], BF16)
        for n in range(2):
            bc_ps = ps_gate.tile([128, 512], F32, tag="gps")
            for k in range(KD):
                nc.tensor.matmul(bc_ps[0:1, :],
                                 finalT_bf[:, k:k + 1],
                                 wbc_bf[:, k, n * 512:(n + 1) * 512],
                                 start=(k == 0), stop=(k == KD - 1))
            bc_half = smalls.tile([1, 512], F32, tag="row")
            if not triv["broadcast_bias"]:
                bbh = smalls.tile([1, 512], F32, tag="brow2")
                nc.sync.dma_start(
                    out=bbh[:],
                    in_=w["broadcast_bias"].ap()[n * 512:(n + 1) * 512]
                        .rearrange("(a d) -> a d", a=1))
                nc.vector.tensor_add(bc_half[:], bc_ps[0:1, :], bbh[:])
            else:
                nc.vector.tensor_copy(bc_half[:], bc_ps[0:1, :])
            bcb_ps = ps_gate.tile([128, 512], F32, tag="gps")
            nc.tensor.matmul(bcb_ps[:], ones_row[:], bc_half[:], start=True, stop=True)
            nc.vector.tensor_copy(bc_bf[:, n * 512:(n + 1) * 512], bcb_ps[:])
        if "d_bcbf" in dbg:
            nc.gpsimd.dma_start(out=dbg["d_bcbf"].ap(), in_=bc_bf[:])
        if "d_gate0" in dbg:
            nc.gpsimd.dma_start(out=dbg["d_gate0"].ap(), in_=gate_bf[:, 0, :])

        gamma_out_b = beta_out_b = None
        if not triv["output_norm"]:
            fin1 = ctx.enter_context(tc.tile_pool(name="fin1", bufs=1))
            gamma_out_b = fin1.tile([128, D], F32)
            nc.sync.dma_start(out=gamma_out_b[:], in_=_bc0(w["output_norm_gamma"].ap()))
            beta_out_b = fin1.tile([128, D], F32)
            nc.sync.dma_start(out=beta_out_b[:], in_=_bc0(w["output_norm_beta"].ap()))

        # ---------------- final: out = LN(h + gate*bc) ----------------
        # pass A: pre (overwrites gate_bf) + row sums / sumsq
        rs_all = singles.tile([128, ML], F32)
        rq_all = singles.tile([128, ML], F32)
        for m in range(ML):
            p1 = fin.tile([128, D], BF16, tag="p1")
            nc.gpsimd.tensor_mul(p1[:], gate_bf[:, m, :], bc_bf[:])
            nc.vector.scalar_tensor_tensor(gate_bf[:, m, :], p1[:], 1.0,
                                           h_bf[:, m, :],
                                           op0=ALU.mult, op1=ALU.add,
                                           accum_out=rs_all[:, m:m + 1])
            sqs = fin.tile([128, D], BF16, tag="sqs")
            nc.scalar.activation(sqs[:], gate_bf[:, m, :], AF.Square,
                                 accum_out=rq_all[:, m:m + 1])
        # pass B: batched stats; rstd = exp(-0.5 ln(var + eps))
        mv_all = singles.tile([128, ML], F32)
        nc.vector.tensor_scalar(mv_all[:], rs_all[:], 1.0 / D, None, op0=ALU.mult)
        m2_all = smalls.tile([128, ML], F32, tag="m2all")
        nc.vector.tensor_mul(m2_all[:], mv_all[:], mv_all[:])
        var_all = smalls.tile([128, ML], F32, tag="varall")
        nc.vector.scalar_tensor_tensor(var_all[:], rq_all[:], 1.0 / D, m2_all[:],
                                       op0=ALU.mult, op1=ALU.subtract)
        lv_all = smalls.tile([128, ML], F32, tag="lvall")
        nc.scalar.activation(lv_all[:], var_all[:], AF.Ln, bias=eps_col[:])
        rstd_all = singles.tile([128, ML], F32)
        nc.scalar.activation(rstd_all[:], lv_all[:], AF.Exp, scale=-0.5)
        # pass C: normalize + store
        for m in range(ML):
            o = fin.tile([128, D], BF16, tag="o")
            nc.vector.tensor_scalar(o[:], gate_bf[:, m, :],
                                    mv_all[:, m:m + 1], rstd_all[:, m:m + 1],
                                    op0=ALU.subtract, op1=ALU.mult)
            if gamma_out_b is not None:
                of = fin.tile([128, D], F32, tag="of")
                nc.vector.tensor_mul(of[:], o[:], gamma_out_b[:])
                nc.vector.tensor_add(of[:], of[:], beta_out_b[:])
                nc.vector.tensor_copy(o[:], of[:])
            nc.sync.dma_start(out=out_ext.ap()[m * 128:(m + 1) * 128, :], in_=o[:])


def _triv_flags(inputs):
    def ones(x):
        return bool(np.all(np.asarray(x) == 1.0))

    def zeros(x):
        return bool(np.all(np.asarray(x) == 0.0))

    return {
        "input_norm": ones(inputs["input_norm_gamma"]) and zeros(inputs["input_norm_beta"]),
        "projector_norm": ones(inputs["projector_norm_gamma"]) and zeros(inputs["projector_norm_beta"]),
        "output_norm": ones(inputs["output_norm_gamma"]) and zeros(inputs["output_norm_beta"]),
        "aggregator_bias": zeros(inputs["aggregator_bias"]),
        "projector_dense1_bias": zeros(inputs["projector_dense1_bias"]),
        "projector_dense2_bias": zeros(inputs["projector_dense2_bias"]),
        "broadcast_bias": zeros(inputs["broadcast_bias"]),
        "gate_bias": zeros(inputs["gate_bias"]),
    }


_GRAPH_CACHE = {}


def prep_in_maps(inputs):
    """Per-core input maps: big tensors host-cast to bf16; W1/W2/b1 sliced
    along H per core; bsel is the per-core one-hot batch selector."""
    import ml_dtypes
    bf = ml_dtypes.bfloat16
    hs = np.ascontiguousarray(
        np.asarray(inputs["hidden_states"], dtype=np.float32).astype(bf))
    assert hs.shape == (B, L, D)
    w1 = np.asarray(inputs["projector_dense1_weight"], np.float32).astype(bf)
    w2 = np.asarray(inputs["projector_dense2_weight"], np.float32).astype(bf)
    b1 = np.asarray(inputs["projector_dense1_bias"], np.float32)
    full = {}
    for n in ("aggregator_weight", "broadcast_weight", "gate_weight"):
        full[n] = np.ascontiguousarray(
            np.asarray(inputs[n], np.float32).astype(bf))
    for n in ("input_norm_gamma", "input_norm_beta", "aggregator_bias",
              "projector_norm_gamma", "projector_norm_beta",
              "projector_dense2_bias", "broadcast_bias", "gate_bias",
              "output_norm_gamma", "output_norm_beta"):
        full[n] = np.ascontiguousarray(np.asarray(inputs[n], np.float32))
    in_maps = []
    for r in range(B):
        sl = slice(r * HSH, (r + 1) * HSH)
        m = {"hidden_states": np.ascontiguousarray(hs[r]),
             "projector_dense1_weight": np.ascontiguousarray(w1[:, sl]),
             "projector_dense2_weight": np.ascontiguousarray(w2[sl, :]),
             "projector_dense1_bias": np.ascontiguousarray(b1[sl]),
             "bsel": np.eye(B, dtype=np.float32)[r].copy()}
        m.update(full)
        in_maps.append(m)
    return in_maps


def kernel(**inputs):
    triv = _triv_flags(inputs)
    key = tuple(sorted(triv.items()))
    if key not in _GRAPH_CACHE:
        _GRAPH_CACHE[key] = build_graph(triv)
    nc = _GRAPH_CACHE[key]
    in_maps = prep_in_maps(inputs)
    res = run_bass_kernel_spmd(nc, in_maps, core_ids=list(range(B)))
    out = np.stack([np.asarray(res.results[b]["out"], dtype=np.float32)
                    for b in range(B)], axis=0)
    return out
